# revision 13
# baseline (speedup 1.0000x reference)
"""CSNN LIF-scan kernel for Trainium2, 8 NeuronCores.

reference computes:
    cur = x @ W.T + b                      # [128, 10000]
    scan t=0..49:  reset = (mem > 1); mem = 0.95*mem + cur - reset
                   spk = (mem > 1)
    returns spk_rec, mem_rec               # each [50, 128, 10000] f32

Sharding: model-parallel over the neuron axis (10000 = 8 x 1250). Each core
keeps batch=128 on SBUF partitions so every output step DMAs as contiguous
rows, and runs the full T=50 scan on its 1250-neuron slice. x is replicated;
W/b are sliced per core. The bias is folded into the matmul as an extra
contraction row (xT row 1000 == 1.0, wT row 1000 == b).
"""

import sys

for _p in ("/opt/trn_rl_repo", "/root/.axon_site/_ro/trn_rl_repo"):
    if _p not in sys.path:
        sys.path.append(_p)

import numpy as np

import concourse.bass as bass
import concourse.tile as tile
from concourse import mybir

F32 = mybir.dt.float32
U8 = mybir.dt.uint8

N_CORES = 8
B = 128          # batch (SBUF partitions)
AXON = 1000      # contraction dim
K_PAD = 1024     # padded contraction (8 x 128); row 1000 carries the bias
N_TOTAL = 10000
NL = N_TOTAL // N_CORES  # 1250 neurons per core
T = 50
BETA = 0.95
THRESH = 1.0

# matmul free-dim chunks (PSUM bank holds 512 f32)
MM_CHUNKS = [(0, 512), (512, 1024), (1024, 1250)]
# spike-compare column split: ScalarE computes Relu(Sign(mem-1)) on the
# first CA columns (exact: mem-1 is Sterbenz-exact in [0.5,2], and the sign
# is all the compare needs); DVE does is_gt on the rest. Balances the two
# engines so the compare is off DVE's critical path.
CA = 950


def _split_excess_waits(bir: dict) -> int:
    """walrus in this env lowers at most ONE sync-wait per instruction, but
    Tile emits several. Move extras onto injected EventSemaphore carriers
    placed just before the instruction on the same engine."""
    n_split = [0]

    def fix_block(block):
        for inner in block.get("blocks", []):
            fix_block(inner)
        insts = block.get("instructions")
        if not insts:
            return
        new_insts = []
        for inst in insts:
            si = inst.get("sync_info")
            waits = (si or {}).get("on_wait", [])
            if len(waits) > 1:
                for w in waits[:-1]:
                    n_split[0] += 1
                    new_insts.append(
                        {
                            "debug": inst.get("debug", 0),
                            "engine": inst["engine"],
                            "ins": [],
                            "name": f"I-wsplit-{n_split[0]}",
                            "opcode": "EventSemaphore",
                            "outs": [],
                            "sync_info": {"on_update": [], "on_wait": [w]},
                        }
                    )
                si["on_wait"] = [waits[-1]]
            new_insts.append(inst)
        block["instructions"] = new_insts

    for fn in bir.get("functions", []):
        fix_block(fn)
    return n_split[0]


def _patch_serialization(nc: bass.Bass) -> bass.Bass:
    import json as _json
    import types as _types

    orig = nc.to_json_bytes

    def to_json_bytes(self):
        bir = _json.loads(orig())
        _split_excess_waits(bir)
        return _json.dumps(bir).encode()

    nc.to_json_bytes = _types.MethodType(to_json_bytes, nc)
    return nc


def _build_program() -> bass.Bass:
    from contextlib import ExitStack

    nc = bass.Bass()
    xT = nc.dram_tensor("xT", [K_PAD, B], F32, kind="ExternalInput")
    wT = nc.dram_tensor("wT", [K_PAD, NL], F32, kind="ExternalInput")
    # spikes are exactly 0/1: ship them as uint8 (4x less DMA) and upcast on
    # the host
    spk_rec = nc.dram_tensor("spk_rec", [T, B, NL], U8, kind="ExternalOutput")
    mem_rec = nc.dram_tensor("mem_rec", [T, B, NL], F32, kind="ExternalOutput")

    KT = K_PAD // 128  # 8 contraction tiles

    with tile.TileContext(nc) as tc, ExitStack() as ctx:
        xpool = ctx.enter_context(tc.tile_pool(name="xp", bufs=KT))
        wpool = ctx.enter_context(tc.tile_pool(name="wp", bufs=KT))
        curp = ctx.enter_context(tc.tile_pool(name="curp", bufs=1))
        psum = ctx.enter_context(tc.tile_pool(name="psum", bufs=3, space="PSUM"))
        memp = ctx.enter_context(tc.tile_pool(name="memp", bufs=6))
        spkp = ctx.enter_context(tc.tile_pool(name="spkp", bufs=6))
        tmpp = ctx.enter_context(tc.tile_pool(name="tmpp", bufs=4))

        x_tiles = []
        w_tiles = []
        for k in range(KT):
            xt = xpool.tile([128, B], F32, tag="x")
            nc.sync.dma_start(out=xt, in_=xT[k * 128 : (k + 1) * 128, :])
            x_tiles.append(xt)
        for k in range(KT):
            wt = wpool.tile([128, NL], F32, tag="w")
            # alternate HWDGE rings so the weight load isn't one-FIFO bound
            eng = nc.sync if k % 2 == 0 else nc.scalar
            eng.dma_start(out=wt, in_=wT[k * 128 : (k + 1) * 128, :])
            w_tiles.append(wt)

        # cur = x @ W.T + b, accumulated over 8 K-tiles per free-dim chunk
        cur = curp.tile([B, NL], F32)
        for n0, n1 in MM_CHUNKS:
            ps = psum.tile([B, n1 - n0], F32, tag="ps")
            for k in range(KT):
                nc.tensor.matmul(
                    ps,
                    x_tiles[k],
                    w_tiles[k][:, n0:n1],
                    start=(k == 0),
                    stop=(k == KT - 1),
                )
            nc.scalar.copy(out=cur[:, n0:n1], in_=ps)

        neg_thresh = curp.tile([B, 1], F32, tag="negth")
        nc.vector.memset(neg_thresh, -THRESH)

        # LIF scan, full row per step. spk = (mem > 1) is computed split:
        # ScalarE does cols [0:CA) via Relu(Sign(mem-1)), DVE the rest.
        def compare_into(s, m):
            nc.scalar.activation(
                out=s[:, :CA], in_=m[:, :CA],
                func=mybir.ActivationFunctionType.Sign, bias=neg_thresh, scale=1.0,
            )
            nc.scalar.activation(
                out=s[:, :CA], in_=s[:, :CA],
                func=mybir.ActivationFunctionType.Relu,
            )
            nc.vector.tensor_scalar(
                out=s[:, CA:], in0=m[:, CA:], scalar1=THRESH, scalar2=None,
                op0=mybir.AluOpType.is_gt,
            )

        # t = 0: mem1 = cur, spk1 = (cur > 1)
        nc.sync.dma_start(out=mem_rec[0], in_=cur)
        s0 = spkp.tile([B, NL], U8, tag="spk")
        compare_into(s0, cur)
        nc.scalar.dma_start(out=spk_rec[0], in_=s0)
        mem_state = cur
        spk_state = s0

        for t in range(1, T):
            u = tmpp.tile([B, NL], F32, tag="u")
            nc.vector.scalar_tensor_tensor(
                out=u, in0=mem_state, scalar=BETA, in1=cur,
                op0=mybir.AluOpType.mult, op1=mybir.AluOpType.add,
            )
            # m = u - spk, phrased as (spk * -1) + u: scalar_tensor_tensor
            # runs in the DVE 2x perf mode while plain tensor_tensor is 1x
            m = memp.tile([B, NL], F32, tag="mem")
            nc.vector.scalar_tensor_tensor(
                out=m, in0=spk_state, scalar=-1.0, in1=u,
                op0=mybir.AluOpType.mult, op1=mybir.AluOpType.add,
            )
            s = spkp.tile([B, NL], U8, tag="spk")
            compare_into(s, m)
            nc.sync.dma_start(out=mem_rec[t], in_=m)
            # spk goes out on the ACT HWDGE ring so the two output streams
            # don't share one FIFO
            nc.scalar.dma_start(out=spk_rec[t], in_=s)
            mem_state = m
            spk_state = s

    return _patch_serialization(nc)


_NC_CACHE = None


def _get_program() -> bass.Bass:
    global _NC_CACHE
    if _NC_CACHE is None:
        _NC_CACHE = _build_program()
    return _NC_CACHE


def _prep_inputs(x: np.ndarray, W: np.ndarray, b: np.ndarray):
    x = np.asarray(x, dtype=np.float32)
    W = np.asarray(W, dtype=np.float32)
    b = np.asarray(b, dtype=np.float32)
    xT = np.zeros((K_PAD, B), dtype=np.float32)
    xT[:AXON] = x.T
    xT[AXON] = 1.0  # bias row
    in_maps = []
    for c in range(N_CORES):
        lo, hi = c * NL, (c + 1) * NL
        wT = np.zeros((K_PAD, NL), dtype=np.float32)
        wT[:AXON] = W[lo:hi].T
        wT[AXON] = b[lo:hi]
        in_maps.append({"xT": xT, "wT": np.ascontiguousarray(wT)})
    return in_maps


def run(x, W, b, trace: bool = False):
    """Run the kernel; returns ((spk_rec, mem_rec), BassKernelResults)."""
    from concourse.bass_utils import run_bass_kernel_spmd

    nc = _get_program()
    in_maps = _prep_inputs(x, W, b)
    res = run_bass_kernel_spmd(
        nc, in_maps, list(range(N_CORES)), trace=trace
    )
    spk = np.concatenate(
        [res.results[c]["spk_rec"] for c in range(N_CORES)], axis=2
    ).astype(np.float32)
    mem = np.concatenate([res.results[c]["mem_rec"] for c in range(N_CORES)], axis=2)
    return (spk, mem), res


def kernel(x: np.ndarray, W: np.ndarray, b: np.ndarray):
    (spk, mem), _ = run(x, W, b)
    return spk, mem
